# revision 25
# baseline (speedup 1.0000x reference)
"""Trainium2 Bass kernel for a custom attention block (qkv-proj + LN(q,k) +
RoPE + causal attention + out-proj), distributed over 8 NeuronCores.

Sharding: 2 cores per batch (B=4). Core role r=c%2 takes q-token blocks
{0,3} (r=0) or {1,2} (r=1) of 512 tokens; every core computes K/V for the
full 2048-token sequence of its batch (no collectives). The compiled
program is identical on all cores; all per-core differences are input
data (sliced x^T, cos/sin tables, causal masks).

v2 (bf16): all matmul operands are bf16 (fp32 PSUM accumulation), which
enables fast weight load and 1024-wide moving operands and halves DMA
traffic vs fp32. K stays SBUF-resident from projection through attention
(8 MB) so attention never re-reads K from HBM; V and q round-trip
through DRAM scratch with deep prefetch. Weights stream from HBM once
per 1024-token x pair. Scores for two kv-chunks share one PSUM tile so
exp runs on [128,1024] activations (ACT at ~1 elem/cycle/lane is the
attention co-bottleneck). LN variance uses Square + ones-matmul
partition reduction with rsqrt computed as Exp(-0.5*Ln(var+eps));
softmax is exp(s-8) with data-driven causal mask multiplies and a
ones-matmul denominator. LN-apply/rope DVE work for each phase is
emitted interleaved into the NEXT phase's matmul stream so the in-order
PE never head-of-line blocks on DVE.
"""

import math

import numpy as np

import concourse.bass as bass
import concourse.mybir as mybir
import concourse.tile as tile
from concourse import bacc
from concourse.bass import ds, ts

F32 = mybir.dt.float32
BF16 = mybir.dt.bfloat16
AF = mybir.ActivationFunctionType
OP = mybir.AluOpType

P = 128
HD = 128

FULL_CFG = dict(
    D=2048,           # model dim (contraction dim for projections)
    S=2048,           # kv tokens per core (full sequence of its batch)
    NQTOK=1024,       # q tokens per core
    QT=512,           # attention q-tile width (moving dim)
    slots=(8, 16),    # kv 128-chunks visited per q-tile
    masked=(tuple(range(0, 8)), tuple(range(8, 16))),  # slots that get a mask
    EXP_BIAS=8.0,
    EPS=1e-5,
)


def build_program(cfg):
    D = cfg["D"]
    S = cfg["S"]
    NQTOK = cfg["NQTOK"]
    QT = cfg["QT"]
    slots = cfg["slots"]
    masked = cfg["masked"]
    EXP_BIAS = cfg["EXP_BIAS"]
    EPS = cfg["EPS"]

    NH = D // HD              # heads == e-chunks per q (and per k)
    DC = D // P               # contraction chunks
    NQ = NQTOK // QT          # q tiles
    KC = S // P               # kv chunks
    MAXM = max(len(m) for m in masked)

    nc = bacc.Bacc("TRN2", target_bir_lowering=False, debug=False)

    # ---- I/O ----
    xT = nc.dram_tensor("xT", [D, S], BF16, kind="ExternalInput").ap()
    xTq = nc.dram_tensor("xTq", [D, NQTOK], BF16, kind="ExternalInput").ap()
    wqkT = nc.dram_tensor("wqkT", [2 * NH, P, DC, P], BF16,
                          kind="ExternalInput").ap()
    wvTt = nc.dram_tensor("wvTt", [NH, P, DC, P], BF16,
                          kind="ExternalInput").ap()
    ident_i = nc.dram_tensor("ident", [P, P], BF16,
                             kind="ExternalInput").ap()
    woTt = nc.dram_tensor("woTt", [DC, P, NH, P], BF16,
                          kind="ExternalInput").ap()
    cosq_i = nc.dram_tensor("cosq", [HD, NQTOK], BF16,
                            kind="ExternalInput").ap()
    sinq_i = nc.dram_tensor("sinqn", [HD, NQTOK], BF16,
                            kind="ExternalInput").ap()
    cosk_i = nc.dram_tensor("cosk", [HD, S], BF16, kind="ExternalInput").ap()
    sink_i = nc.dram_tensor("sinkn", [HD, S], BF16, kind="ExternalInput").ap()
    rotm_i = nc.dram_tensor("rotm", [P, P], BF16, kind="ExternalInput").ap()
    onesc_i = nc.dram_tensor("onesc", [P, 1], BF16, kind="ExternalInput").ap()
    onesr_i = nc.dram_tensor("onesr", [1, P], BF16, kind="ExternalInput").ap()
    gq_i = nc.dram_tensor("gq", [P, NH], F32, kind="ExternalInput").ap()
    bq_i = nc.dram_tensor("bq", [P, NH], F32, kind="ExternalInput").ap()
    gk_i = nc.dram_tensor("gk", [P, NH], F32, kind="ExternalInput").ap()
    bk_i = nc.dram_tensor("bk", [P, NH], F32, kind="ExternalInput").ap()
    masks_i = nc.dram_tensor("masks", [NQ, P, MAXM, QT], BF16,
                             kind="ExternalInput").ap()
    out_t = nc.dram_tensor("out", [D, NQTOK], F32, kind="ExternalOutput").ap()

    with tile.TileContext(nc) as tc:
        import contextlib

        ctx = contextlib.ExitStack()
        with ctx:
            sb = ctx.enter_context(tc.tile_pool(name="sb", bufs=1))
            psum = ctx.enter_context(tc.tile_pool(name="ps", bufs=1,
                                                  space="PSUM"))
            dram = ctx.enter_context(tc.tile_pool(name="dram", bufs=1,
                                                  space="DRAM"))

            # ---- DRAM scratch ----
            qts = dram.tile([NQ, NH, P, QT], BF16, tag="qts", name="qts")
            vts = dram.tile([NH, S, HD], BF16, tag="vts", name="vts")

            # ---- persistent SBUF: K slabs (one per 1024-token pair, so
            # attention tile 0 never false-depends on pair-1 writes) ----
            kslabs = [
                sb.tile([P, NH, 1024], BF16, tag=f"kslab{pr}",
                        name=f"kslab{pr}")
                for pr in range(2)
            ]

            ones_col = sb.tile([P, 1], BF16, tag="ones_col", name="ones_col")
            nc.sync.dma_start(ones_col, onesc_i)
            ones_row = sb.tile([1, P], BF16, tag="ones_row", name="ones_row")
            nc.sync.dma_start(ones_row, onesr_i)
            eps1 = sb.tile([1, 1], F32, tag="eps1", name="eps1")
            nc.vector.memset(eps1, EPS)
            zero1 = sb.tile([1, 1], F32, tag="zero1", name="zero1")
            nc.vector.memset(zero1, 0.0)
            nege = sb.tile([P, 1], F32, tag="nege", name="nege")
            nc.vector.memset(nege, -EXP_BIAS)
            rotm = sb.tile([P, P], BF16, tag="rotm", name="rotm")
            nc.sync.dma_start(rotm, rotm_i)
            ident = sb.tile([P, P], BF16, tag="ident", name="ident")
            nc.sync.dma_start(ident, ident_i)
            gq = sb.tile([P, NH], F32, tag="gq", name="gq")
            nc.sync.dma_start(gq, gq_i)
            bq = sb.tile([P, NH], F32, tag="bq", name="bq")
            nc.sync.dma_start(bq, bq_i)
            gk = sb.tile([P, NH], F32, tag="gk", name="gk")
            nc.sync.dma_start(gk, gk_i)
            bk = sb.tile([P, NH], F32, tag="bk", name="bk")
            nc.sync.dma_start(bk, bk_i)

            proj_ctx = contextlib.ExitStack()
            proj = proj_ctx.enter_context(tc.tile_pool(name="proj", bufs=1))

            def stats_finish(pstat):
                """pstat [1, QT] sumsq psum -> [P, QT] bf16 rsig broadcast."""
                lnv = proj.tile([1, QT], F32, tag="stats_sb", bufs=4,
                                name="lnv")
                nc.scalar.activation(lnv, pstat, AF.Ln, scale=1.0 / D,
                                     bias=eps1)
                rsig = proj.tile([1, QT], BF16, tag="stats_sb", bufs=4,
                                 name="rsig")
                nc.scalar.activation(rsig, lnv, AF.Exp, bias=zero1,
                                     scale=-0.5)
                ps_rep = psum.tile([P, QT], F32, tag="sc", bufs=2,
                                   name="ps_rep")
                nc.tensor.matmul(ps_rep, lhsT=ones_row, rhs=rsig)
                rsb = proj.tile([P, QT], BF16, tag="rsb", bufs=4, name="rsb")
                nc.vector.tensor_copy(rsb, ps_rep)
                return rsb

            def ln_apply(sl, rsb, g_sb, b_sb, ec):
                """In-place LN affine on a [P, QT] bf16 slab slice."""
                nc.vector.tensor_tensor(sl, sl, rsb, op=OP.mult)
                nc.vector.tensor_scalar(
                    sl, sl,
                    scalar1=g_sb[:, ds(ec, 1)],
                    scalar2=b_sb[:, ds(ec, 1)],
                    op0=OP.mult, op1=OP.add,
                )

            def rope_apply(sl, cos_sl, sin_sl):
                """In-place rope on a [P, QT] bf16 slab slice."""
                ps_rot = psum.tile([P, QT], F32, tag="sc", bufs=2,
                                   name="ps_rot")
                nc.tensor.matmul(ps_rot, lhsT=rotm, rhs=sl)
                tmp = proj.tile([P, QT], BF16, tag="tmp", bufs=2, name="tmp")
                nc.vector.tensor_tensor(tmp, ps_rot, sin_sl, op=OP.mult)
                nc.vector.tensor_tensor(sl, sl, cos_sl, op=OP.mult)
                nc.vector.tensor_tensor(sl, sl, tmp, op=OP.add)

            # ---------------- emission helpers -----------------------
            # Deferred-work queues: each entry is a closure emitting the
            # LN+rope (DVE + 1 small PE matmul) for one head chunk. They
            # are drained interleaved into the next phase's matmul stream
            # so the PE never stalls on DVE and the DVE FIFO never blocks
            # a PSUM-slot reuse.
            def kproj_pair(pr, drain):
                """k projection for 1024-token pair pr; returns (rsbs,
                deferred ln+rope closures). drain: list of closures to
                interleave (one per ec)."""
                tok0 = pr * 1024
                kslab = kslabs[pr]
                xb = proj.tile([P, DC, 1024], BF16, tag="xb", bufs=1,
                               name="xb")
                nc.sync.dma_start(
                    xb, xT[:, ds(tok0, 1024)].rearrange(
                        "(d p) n -> p d n", p=P
                    )
                )
                cosk = proj.tile([HD, 1024], BF16, tag="cs2", bufs=4,
                                 name="cosk")
                nc.sync.dma_start(cosk, cosk_i[:, ds(tok0, 1024)])
                sink = proj.tile([HD, 1024], BF16, tag="cs2", bufs=4,
                                 name="sink")
                nc.sync.dma_start(sink, sink_i[:, ds(tok0, 1024)])
                pstats = [
                    psum.tile([1, QT], F32, tag="st", bufs=2, name="pstat_k")
                    for _ in range(2)
                ]
                for ec in range(NH):
                    w = proj.tile([P, DC, P], BF16, tag="w", bufs=4,
                                  name="wk")
                    nc.sync.dma_start(w, wqkT[NH + ec])
                    psk = psum.tile([P, 1024], F32, tag="pair", bufs=2,
                                    name="psk")
                    for d in range(DC):
                        for half in range(2):
                            nc.tensor.matmul(
                                psk[:, ds(half * QT, QT)], lhsT=w[:, d],
                                rhs=xb[:, d, ds(half * QT, QT)],
                                start=(d == 0), stop=(d == DC - 1),
                            )
                    nc.vector.tensor_copy(kslab[:, ec, :], psk)
                    sq = proj.tile([P, 1024], BF16, tag="sq", bufs=2,
                                   name="sqk")
                    nc.scalar.square(sq, psk)
                    for half in range(2):
                        nc.tensor.matmul(pstats[half], lhsT=ones_col,
                                         rhs=sq[:, ds(half * QT, QT)],
                                         start=(ec == 0), stop=(ec == NH - 1))
                    if ec < len(drain):
                        drain[ec]()
                rsbs = [stats_finish(pstats[half]) for half in range(2)]
                return xb, rsbs, cosk, sink

            def k_lnrope_closures(pr, rsbs, cosk, sink):
                kslab = kslabs[pr]
                out = []
                for ec in range(NH):
                    def fn(ec=ec):
                        for half in range(2):
                            sl = kslab[:, ec, ds(half * QT, QT)]
                            ln_apply(sl, rsbs[half], gk, bk, ec)
                            rope_apply(sl, cosk[:, ds(half * QT, QT)],
                                       sink[:, ds(half * QT, QT)])
                    out.append(fn)
                return out

            def vproj_pair(pr, xb, drain):
                """v projection for pair pr using resident xb; writes vts
                (token-major) via per-head feature-major matmuls + PE block
                transposes. drain: closures interleaved per head."""
                for ecv in range(NH):
                    wvc = proj.tile([P, DC, P], BF16, tag="w", bufs=4,
                                    name="wvc")
                    nc.sync.dma_start(wvc, wvTt[ecv])
                    psvf = psum.tile([P, 1024], F32, tag="pair", bufs=2,
                                     name="psvf")
                    for d in range(DC):
                        for half in range(2):
                            nc.tensor.matmul(
                                psvf[:, ds(half * QT, QT)], lhsT=wvc[:, d],
                                rhs=xb[:, d, ds(half * QT, QT)],
                                start=(d == 0), stop=(d == DC - 1),
                            )
                    vfsb = proj.tile([P, 1024], BF16, tag="vsb", bufs=3,
                                     name="vfsb")
                    nc.vector.tensor_copy(vfsb, psvf)
                    vtsb = proj.tile([P, 8, HD], BF16, tag="vtsb", bufs=2,
                                     name="vtsb")
                    for tc8 in range(8):
                        pst = psum.tile([P, P], BF16, tag="st", bufs=2,
                                        name="pst")
                        nc.tensor.transpose(pst, vfsb[:, ds(tc8 * P, P)],
                                            ident)
                        nc.vector.tensor_copy(vtsb[:, tc8], pst)
                    nc.sync.dma_start(
                        vts[ecv, ds(pr * 1024, 1024), :].rearrange(
                            "(kc p) hd -> p kc hd", p=P
                        ),
                        vtsb,
                    )
                    for di in (2 * ecv, 2 * ecv + 1):
                        if di < len(drain):
                            drain[di]()

            def qproj_tile(t, drain):
                """q projection for 512-token tile t into holdq; stats
                finished; returns (holdq, rsb)."""
                xq = proj.tile([P, DC, QT], BF16, tag="xq", bufs=1,
                               name="xq")
                nc.sync.dma_start(
                    xq, xTq[:, ds(t * QT, QT)].rearrange(
                        "(d p) n -> p d n", p=P
                    )
                )
                cosq = proj.tile([HD, QT], BF16, tag="cs", bufs=4,
                                 name="cosq")
                nc.sync.dma_start(cosq, cosq_i[:, ds(t * QT, QT)])
                sinq = proj.tile([HD, QT], BF16, tag="cs", bufs=4,
                                 name="sinq")
                nc.sync.dma_start(sinq, sinq_i[:, ds(t * QT, QT)])
                holdq = proj.tile([P, NH, QT], BF16, tag="holdq", bufs=2,
                                  name="holdq")
                pstat = psum.tile([1, QT], F32, tag="st", bufs=2,
                                  name="pstat_q")
                for ec in range(NH):
                    w = proj.tile([P, DC, P], BF16, tag="w", bufs=4,
                                  name="wq")
                    nc.sync.dma_start(w, wqkT[ec])
                    psq = psum.tile([P, QT], F32, tag="sc", bufs=2,
                                    name="psq")
                    for d in range(DC):
                        nc.tensor.matmul(psq, lhsT=w[:, d], rhs=xq[:, d],
                                         start=(d == 0), stop=(d == DC - 1))
                    nc.vector.tensor_copy(holdq[:, ec], psq)
                    sq = proj.tile([P, QT], BF16, tag="sq", bufs=2,
                                   name="sqq")
                    nc.scalar.square(sq, psq)
                    nc.tensor.matmul(pstat, lhsT=ones_col, rhs=sq,
                                     start=(ec == 0), stop=(ec == NH - 1))
                    if ec < len(drain):
                        drain[ec]()
                rsb = stats_finish(pstat)
                return holdq, rsb, cosq, sinq

            def q_lnrope_closures(t, holdq, rsb, cosq, sinq):
                out = []
                for ec in range(NH):
                    def fn(ec=ec, last=(ec == NH - 1)):
                        sl = holdq[:, ec]
                        ln_apply(sl, rsb, gq, bq, ec)
                        rope_apply(sl, cosq, sinq)
                        if last:
                            nc.sync.dma_start(
                                qts[t].rearrange("h p q -> p h q"), holdq
                            )
                    out.append(fn)
                return out

            # ---------------- projection schedule ---------------------
            # each phase's LN/rope DVE work drains interleaved into the
            # next phase's matmul stream
            hq0, rsbq0, cq0, sq0 = qproj_tile(0, drain=[])
            lnq0 = q_lnrope_closures(0, hq0, rsbq0, cq0, sq0)
            xb0, rsbs0, ck0, sk0 = kproj_pair(0, drain=lnq0)
            ln0 = k_lnrope_closures(0, rsbs0, ck0, sk0)
            vproj_pair(0, xb0, drain=ln0)

            hq1, rsbq1, cq1, sq1 = qproj_tile(1, drain=[])
            lnq1 = q_lnrope_closures(1, hq1, rsbq1, cq1, sq1)
            xb1, rsbs1, ck1, sk1 = kproj_pair(1, drain=lnq1)
            ln1 = k_lnrope_closures(1, rsbs1, ck1, sk1)
            vproj_pair(1, xb1, drain=ln1)

            proj_ctx.close()

            # ============ attention + out-projection ===========
            attn_ctx = contextlib.ExitStack()
            attn = attn_ctx.enter_context(tc.tile_pool(name="attn", bufs=1))
            ot = attn.tile([P, NH, NQTOK], BF16, tag="ot", name="ot")
            PRE = 2  # score pair lookahead

            for t in range(NQ):
                mt = attn.tile([P, MAXM, QT], BF16, tag="masks", bufs=2,
                               name="mt")
                nc.sync.dma_start(mt, masks_i[t])
                mpos = {kc: i for i, kc in enumerate(masked[t])}
                n_slots = slots[t]
                n_pairs = n_slots // 2
                for h in range(NH):
                    qsl = attn.tile([P, QT], BF16, tag="qslab", bufs=3,
                                    name="qsl")
                    nc.sync.dma_start(qsl, qts[t, h])
                    vsl = attn.tile([P, KC, HD], BF16, tag="vslab", bufs=3,
                                    name="vsl")
                    nc.sync.dma_start(
                        vsl[:, :n_slots],
                        vts[h, ds(0, n_slots * P), :].rearrange(
                            "(kc p) hd -> p kc hd", p=P
                        ),
                    )
                    psout = psum.tile([P, QT], F32, tag="sc", bufs=2,
                                      name="psout")
                    psden = psum.tile([1, QT], F32, tag="st", bufs=2,
                                      name="psden")
                    dacc = attn.tile([P, 1024], BF16, tag="dacc", bufs=2,
                                     name="dacc")

                    ps_pairs = {}

                    def issue_scores(p, h=h, qsl=qsl, ps_pairs=ps_pairs):
                        pp = psum.tile([P, 1024], F32, tag="pair", bufs=2,
                                       name="pp")
                        for half in range(2):
                            kc = 2 * p + half
                            nc.tensor.matmul(
                                pp[:, ds(half * QT, QT)],
                                lhsT=kslabs[kc // 8][:, h,
                                                    ds((kc % 8) * P, P)],
                                rhs=qsl,
                            )
                        ps_pairs[p] = pp

                    for p in range(min(PRE, n_pairs)):
                        issue_scores(p)
                    for p in range(n_pairs):
                        if p + PRE < n_pairs:
                            issue_scores(p + PRE)
                        pp = ps_pairs.pop(p)
                        et2 = attn.tile([P, 1024], BF16, tag="exp", bufs=3,
                                        name="et2")
                        nc.scalar.activation(et2, pp, AF.Exp, bias=nege)
                        for half in range(2):
                            kc = 2 * p + half
                            if kc in mpos:
                                nc.vector.tensor_tensor(
                                    et2[:, ds(half * QT, QT)],
                                    et2[:, ds(half * QT, QT)],
                                    mt[:, mpos[kc]], op=OP.mult,
                                )
                        for half in range(2):
                            kc = 2 * p + half
                            nc.tensor.matmul(
                                psout,
                                lhsT=vsl[:, kc],
                                rhs=et2[:, ds(half * QT, QT)],
                                start=(kc == 0), stop=(kc == n_slots - 1),
                            )
                        # denominator: first half of the pairs accumulates
                        # on DVE (dacc), second half goes straight to the
                        # PE psden group — balances DVE vs PE in the
                        # attention inner loop.
                        if p == 0:
                            nc.vector.tensor_copy(dacc, et2)
                        elif p < (n_pairs + 1) // 2:
                            nc.vector.tensor_tensor(dacc, dacc, et2,
                                                    op=OP.add)
                        else:
                            for half in range(2):
                                nc.tensor.matmul(
                                    psden, lhsT=ones_col,
                                    rhs=et2[:, ds(half * QT, QT)],
                                    start=(p == (n_pairs + 1) // 2
                                           and half == 0),
                                    stop=False,
                                )

                    for half in range(2):
                        nc.tensor.matmul(psden, lhsT=ones_col,
                                         rhs=dacc[:, ds(half * QT, QT)],
                                         start=False, stop=(half == 1))
                    rec0 = attn.tile([1, QT], F32, tag="stats_sb", bufs=4,
                                     name="rec0")
                    with nc.allow_low_precision(
                        reason="denominator reciprocal, 18 bits is plenty"
                    ):
                        nc.vector.reciprocal_approx_fast(rec0, psden)
                    rec = attn.tile([1, QT], BF16, tag="stats_sb", bufs=4,
                                    name="rec")
                    nc.scalar.activation(rec, rec0, AF.Copy)
                    psr = psum.tile([P, QT], F32, tag="sc", bufs=2,
                                    name="psr")
                    nc.tensor.matmul(psr, lhsT=ones_row, rhs=rec)
                    rsb = attn.tile([P, QT], BF16, tag="rsbn", bufs=2,
                                    name="rsbn")
                    nc.vector.tensor_copy(rsb, psr)
                    nc.vector.tensor_tensor(ot[:, h, ds(t * QT, QT)], psout,
                                            rsb, op=OP.mult)

            # ---- out-projection over all q tokens ----
            for ec in range(DC):
                woc = attn.tile([P, NH, P], BF16, tag="woc", bufs=3,
                                name="woc")
                nc.sync.dma_start(woc, woTt[ec])
                psf = psum.tile([P, 1024], F32, tag="pair", bufs=2,
                                name="psf")
                for h in range(NH):
                    for half in range(2):
                        nc.tensor.matmul(
                            psf[:, ds(half * QT, QT)], lhsT=woc[:, h],
                            rhs=ot[:, h, ds(half * QT, QT)],
                            start=(h == 0), stop=(h == NH - 1),
                        )
                fsb = attn.tile([P, 1024], F32, tag="fsb", bufs=2, name="fsb")
                nc.vector.tensor_copy(fsb, psf)
                nc.sync.dma_start(out_t[ds(ec * P, P), :], fsb)

            attn_ctx.close()

    nc.compile()
    return nc


# --------------------------------------------------------------------------
# Host-side prep and driver
# --------------------------------------------------------------------------

def _q_blocks(role):
    """q-block indices (each 512 tokens) for a core role."""
    return [0, 3] if role == 0 else [1, 2]


def make_host_data(x, w_in, w_out, q_gamma, q_beta, k_gamma, k_beta, cfg,
                   n_cores=None):
    """Build per-core in_maps (list of dicts) + assembly metadata."""
    import ml_dtypes

    BF = ml_dtypes.bfloat16
    D = cfg["D"]
    S = cfg["S"]
    NQTOK = cfg["NQTOK"]
    QT = cfg["QT"]
    masked = cfg["masked"]
    NH = D // HD
    DC = D // P
    NQ = NQTOK // QT
    MAXM = max(len(m) for m in masked)
    B = x.shape[0]
    if n_cores is None:
        n_cores = 2 * B

    w64 = np.asarray(w_in, np.float64)
    wq = w64[0:D]
    wk = w64[D:2 * D]
    wv = w64[2 * D:3 * D]
    wq_c = wq - wq.mean(axis=0, keepdims=True)
    wk_c = wk - wk.mean(axis=0, keepdims=True)
    wqkT2 = np.concatenate([wq_c.T, wk_c.T], axis=1).astype(BF)
    # pre-tile to [2*NH, P, DC, P]: tile ec -> [p, dc, e] with contiguous rows
    wqkT = np.ascontiguousarray(
        wqkT2.reshape(DC, P, 2 * NH, P).transpose(2, 1, 0, 3)
    )
    wvTt = np.ascontiguousarray(
        wv.T.astype(BF).reshape(DC, P, NH, P).transpose(2, 1, 0, 3)
    )
    woT = np.asarray(w_out, np.float64).T  # [feat, e]
    # [ec, p(feat within head), h, e-col]
    woTt = np.ascontiguousarray(
        woT.reshape(NH, P, DC, P).transpose(2, 1, 0, 3).astype(BF)
    )

    inv = 1.0 / (10000.0 ** (np.arange(0, HD, 2, dtype=np.float64) / HD))
    tpos = np.arange(S, dtype=np.float64)
    fr = np.outer(tpos, inv)
    emb = np.concatenate([fr, fr], axis=-1)  # [S, HD]
    cosT = np.cos(emb).T  # [HD, S]
    sinTn = np.sin(emb).T

    # signed rotate-half permutation, as matmul lhsT:
    # out[p] = sum_{p'} rotmT[p', p] * in[p'] = rot_half(in)[p]
    h2 = HD // 2
    rotmT = np.zeros((P, P), np.float32)
    for p in range(h2):
        rotmT[p + h2, p] = -1.0
    for p in range(h2, HD):
        rotmT[p - h2, p] = 1.0

    scale = 1.0 / math.sqrt(HD)
    gq_a = np.ascontiguousarray(
        (np.asarray(q_gamma, np.float64) * scale).reshape(NH, P).T
    ).astype(np.float32)
    bq_a = np.ascontiguousarray(
        (np.asarray(q_beta, np.float64) * scale).reshape(NH, P).T
    ).astype(np.float32)
    gk_a = np.ascontiguousarray(
        np.asarray(k_gamma, np.float32).reshape(NH, P).T
    )
    bk_a = np.ascontiguousarray(
        np.asarray(k_beta, np.float32).reshape(NH, P).T
    )

    cosk = np.ascontiguousarray(cosT.astype(BF))
    sink = np.ascontiguousarray(sinTn.astype(BF))

    in_maps = []
    meta = []
    cores_per_batch = max(1, n_cores // B)
    for c in range(n_cores):
        b = c // cores_per_batch
        r = c % cores_per_batch
        blocks = _q_blocks(r)
        qtok = np.concatenate(
            [np.arange(bk * 512, (bk + 1) * 512) for bk in blocks]
        )
        xb = np.asarray(x[b], np.float32)  # [S, D]
        xTf = xb.T.astype(BF)              # [D, S]
        xT = np.ascontiguousarray(xTf)
        xTq = np.ascontiguousarray(xTf[:, qtok])
        cosq = np.ascontiguousarray(cosT[:, qtok].astype(BF))
        sinq = np.ascontiguousarray(sinTn[:, qtok].astype(BF))

        masks = np.zeros([NQ, P, MAXM, QT], np.float32)
        for t in range(NQ):
            q_start = blocks[t] * 512
            qq = np.arange(QT)
            kk = np.arange(P)
            for mi, kc in enumerate(masked[t]):
                masks[t, :, mi, :] = (
                    (kc * P + kk[:, None]) <= (q_start + qq[None, :])
                ).astype(np.float32)
        masks = masks.astype(BF)

        in_maps.append(dict(
            xTq=xTq, xT=xT, wqkT=wqkT, wvTt=wvTt, woTt=woTt,
            cosq=cosq, sinqn=sinq, cosk=cosk, sinkn=sink,
            gq=gq_a, bq=bq_a, gk=gk_a, bk=bk_a, masks=masks,
            rotm=rotmT.astype(BF),
            ident=np.eye(P, dtype=np.float32).astype(BF),
            onesc=np.ones((P, 1), BF),
            onesr=np.ones((1, P), BF),
        ))
        meta.append(dict(b=b, qtok=qtok))
    return in_maps, meta


_PROGRAM_CACHE = {}


def _get_program(cfg_key, cfg):
    if cfg_key not in _PROGRAM_CACHE:
        _PROGRAM_CACHE[cfg_key] = build_program(cfg)
    return _PROGRAM_CACHE[cfg_key]


def run_full(x, w_in, w_out, q_gamma, q_beta, k_gamma, k_beta,
             trace=False):
    from concourse.bass_utils import run_bass_kernel_spmd

    cfg = FULL_CFG
    B = x.shape[0]
    n_cores = 2 * B
    in_maps, meta = make_host_data(
        x, w_in, w_out, q_gamma, q_beta, k_gamma, k_beta, cfg,
        n_cores=n_cores,
    )
    nc = _get_program("full", cfg)
    res = run_bass_kernel_spmd(
        nc, in_maps, core_ids=list(range(n_cores)), trace=trace,
    )
    S, D = cfg["S"], cfg["D"]
    out = np.empty((B, S, D), np.float32)
    for c in range(n_cores):
        o = res.results[c]["out"]  # [D, NQTOK]
        out[meta[c]["b"], meta[c]["qtok"], :] = o.T
    return out, res


def kernel(x, w_in, w_out, q_gamma, q_beta, k_gamma, k_beta, n_heads=16,
           **_ignored):
    x = np.asarray(x, np.float32)
    assert int(np.asarray(n_heads)) * HD == x.shape[-1]
    out, _ = run_full(
        np.asarray(x, np.float32),
        np.asarray(w_in, np.float32),
        np.asarray(w_out, np.float32),
        np.asarray(q_gamma, np.float32),
        np.asarray(q_beta, np.float32),
        np.asarray(k_gamma, np.float32),
        np.asarray(k_beta, np.float32),
    )
    return out


# revision 26
# speedup vs baseline: 1.0199x; 1.0199x over previous
"""Trainium2 Bass kernel for a custom attention block (qkv-proj + LN(q,k) +
RoPE + causal attention + out-proj), distributed over 8 NeuronCores.

Sharding: 2 cores per batch (B=4). Core role r=c%2 takes q-token blocks
{0,3} (r=0) or {1,2} (r=1) of 512 tokens; every core computes K/V for the
full 2048-token sequence of its batch (no collectives). The compiled
program is identical on all cores; all per-core differences are input
data (sliced x^T, cos/sin tables, causal masks).

v2 (bf16): all matmul operands are bf16 (fp32 PSUM accumulation), which
enables fast weight load and 1024-wide moving operands and halves DMA
traffic vs fp32. K stays SBUF-resident from projection through attention
(8 MB) so attention never re-reads K from HBM; V and q round-trip
through DRAM scratch with deep prefetch. Weights stream from HBM once
per 1024-token x pair. Scores for two kv-chunks share one PSUM tile so
exp runs on [128,1024] activations (ACT at ~1 elem/cycle/lane is the
attention co-bottleneck). LN variance uses Square + ones-matmul
partition reduction with rsqrt computed as Exp(-0.5*Ln(var+eps));
softmax is exp(s-8) with data-driven causal mask multiplies and a
ones-matmul denominator. LN-apply/rope DVE work for each phase is
emitted interleaved into the NEXT phase's matmul stream so the in-order
PE never head-of-line blocks on DVE.
"""

import math

import numpy as np

import concourse.bass as bass
import concourse.mybir as mybir
import concourse.tile as tile
from concourse import bacc
from concourse.bass import ds, ts

F32 = mybir.dt.float32
BF16 = mybir.dt.bfloat16
AF = mybir.ActivationFunctionType
OP = mybir.AluOpType

P = 128
HD = 128

FULL_CFG = dict(
    D=2048,           # model dim (contraction dim for projections)
    S=2048,           # kv tokens per core (full sequence of its batch)
    NQTOK=1024,       # q tokens per core
    QT=512,           # attention q-tile width (moving dim)
    slots=(8, 16),    # kv 128-chunks visited per q-tile
    masked=(tuple(range(0, 8)), tuple(range(8, 16))),  # slots that get a mask
    EXP_BIAS=8.0,
    EPS=1e-5,
)


def build_program(cfg):
    D = cfg["D"]
    S = cfg["S"]
    NQTOK = cfg["NQTOK"]
    QT = cfg["QT"]
    slots = cfg["slots"]
    masked = cfg["masked"]
    EXP_BIAS = cfg["EXP_BIAS"]
    EPS = cfg["EPS"]

    NH = D // HD              # heads == e-chunks per q (and per k)
    DC = D // P               # contraction chunks
    NQ = NQTOK // QT          # q tiles
    KC = S // P               # kv chunks
    MAXM = max(len(m) for m in masked)

    nc = bacc.Bacc("TRN2", target_bir_lowering=False, debug=False)

    # ---- I/O ----
    xT = nc.dram_tensor("xT", [D, S], BF16, kind="ExternalInput").ap()
    xTq = nc.dram_tensor("xTq", [D, NQTOK], BF16, kind="ExternalInput").ap()
    wqkT = nc.dram_tensor("wqkT", [2 * NH, P, DC, P], BF16,
                          kind="ExternalInput").ap()
    wvTt = nc.dram_tensor("wvTt", [NH, P, DC, P], BF16,
                          kind="ExternalInput").ap()
    ident_i = nc.dram_tensor("ident", [P, P], BF16,
                             kind="ExternalInput").ap()
    woTt = nc.dram_tensor("woTt", [DC, P, NH, P], BF16,
                          kind="ExternalInput").ap()
    cosq_i = nc.dram_tensor("cosq", [HD, NQTOK], BF16,
                            kind="ExternalInput").ap()
    sinq_i = nc.dram_tensor("sinqn", [HD, NQTOK], BF16,
                            kind="ExternalInput").ap()
    cosk_i = nc.dram_tensor("cosk", [HD, S], BF16, kind="ExternalInput").ap()
    sink_i = nc.dram_tensor("sinkn", [HD, S], BF16, kind="ExternalInput").ap()
    rotm_i = nc.dram_tensor("rotm", [P, P], BF16, kind="ExternalInput").ap()
    onesc_i = nc.dram_tensor("onesc", [P, 1], BF16, kind="ExternalInput").ap()
    onesr_i = nc.dram_tensor("onesr", [1, P], BF16, kind="ExternalInput").ap()
    gq_i = nc.dram_tensor("gq", [P, NH], F32, kind="ExternalInput").ap()
    bq_i = nc.dram_tensor("bq", [P, NH], F32, kind="ExternalInput").ap()
    gk_i = nc.dram_tensor("gk", [P, NH], F32, kind="ExternalInput").ap()
    bk_i = nc.dram_tensor("bk", [P, NH], F32, kind="ExternalInput").ap()
    masks_i = nc.dram_tensor("masks", [NQ, P, MAXM, QT], BF16,
                             kind="ExternalInput").ap()
    out_t = nc.dram_tensor("out", [D, NQTOK], F32, kind="ExternalOutput").ap()

    with tile.TileContext(nc) as tc:
        import contextlib

        ctx = contextlib.ExitStack()
        with ctx:
            sb = ctx.enter_context(tc.tile_pool(name="sb", bufs=1))
            psum = ctx.enter_context(tc.tile_pool(name="ps", bufs=1,
                                                  space="PSUM"))
            dram = ctx.enter_context(tc.tile_pool(name="dram", bufs=1,
                                                  space="DRAM"))

            # ---- DRAM scratch ----
            qts = dram.tile([NQ, NH, P, QT], BF16, tag="qts", name="qts")
            vts = dram.tile([NH, S, HD], BF16, tag="vts", name="vts")

            # ---- persistent SBUF: K slabs (one per 1024-token pair, so
            # attention tile 0 never false-depends on pair-1 writes) ----
            kslabs = [
                sb.tile([P, NH, 1024], BF16, tag=f"kslab{pr}",
                        name=f"kslab{pr}")
                for pr in range(2)
            ]

            ones_col = sb.tile([P, 1], BF16, tag="ones_col", name="ones_col")
            nc.sync.dma_start(ones_col, onesc_i)
            ones_row = sb.tile([1, P], BF16, tag="ones_row", name="ones_row")
            nc.sync.dma_start(ones_row, onesr_i)
            eps1 = sb.tile([1, 1], F32, tag="eps1", name="eps1")
            nc.vector.memset(eps1, EPS)
            zero1 = sb.tile([1, 1], F32, tag="zero1", name="zero1")
            nc.vector.memset(zero1, 0.0)
            nege = sb.tile([P, 1], F32, tag="nege", name="nege")
            nc.vector.memset(nege, -EXP_BIAS)
            rotm = sb.tile([P, P], BF16, tag="rotm", name="rotm")
            nc.sync.dma_start(rotm, rotm_i)
            ident = sb.tile([P, P], BF16, tag="ident", name="ident")
            nc.sync.dma_start(ident, ident_i)
            gq = sb.tile([P, NH], F32, tag="gq", name="gq")
            nc.sync.dma_start(gq, gq_i)
            bq = sb.tile([P, NH], F32, tag="bq", name="bq")
            nc.sync.dma_start(bq, bq_i)
            gk = sb.tile([P, NH], F32, tag="gk", name="gk")
            nc.sync.dma_start(gk, gk_i)
            bk = sb.tile([P, NH], F32, tag="bk", name="bk")
            nc.sync.dma_start(bk, bk_i)

            proj_ctx = contextlib.ExitStack()
            proj = proj_ctx.enter_context(tc.tile_pool(name="proj", bufs=1))

            def stats_finish(pstat):
                """pstat [1, QT] sumsq psum -> [P, QT] bf16 rsig broadcast."""
                lnv = proj.tile([1, QT], F32, tag="stats_sb", bufs=4,
                                name="lnv")
                nc.scalar.activation(lnv, pstat, AF.Ln, scale=1.0 / D,
                                     bias=eps1)
                rsig = proj.tile([1, QT], BF16, tag="stats_sb", bufs=4,
                                 name="rsig")
                nc.scalar.activation(rsig, lnv, AF.Exp, bias=zero1,
                                     scale=-0.5)
                ps_rep = psum.tile([P, QT], F32, tag="sc", bufs=2,
                                   name="ps_rep")
                nc.tensor.matmul(ps_rep, lhsT=ones_row, rhs=rsig)
                rsb = proj.tile([P, QT], BF16, tag="rsb", bufs=4, name="rsb")
                nc.vector.tensor_copy(rsb, ps_rep)
                return rsb

            def ln_apply(sl, rsb, g_sb, b_sb, ec):
                """In-place LN affine on a [P, QT] bf16 slab slice."""
                nc.vector.tensor_tensor(sl, sl, rsb, op=OP.mult)
                nc.vector.tensor_scalar(
                    sl, sl,
                    scalar1=g_sb[:, ds(ec, 1)],
                    scalar2=b_sb[:, ds(ec, 1)],
                    op0=OP.mult, op1=OP.add,
                )

            def rope_apply(sl, cos_sl, sin_sl):
                """In-place rope on a [P, QT] bf16 slab slice."""
                ps_rot = psum.tile([P, QT], F32, tag="sc", bufs=2,
                                   name="ps_rot")
                nc.tensor.matmul(ps_rot, lhsT=rotm, rhs=sl)
                tmp = proj.tile([P, QT], BF16, tag="tmp", bufs=2, name="tmp")
                nc.vector.tensor_tensor(tmp, ps_rot, sin_sl, op=OP.mult)
                nc.vector.tensor_tensor(sl, sl, cos_sl, op=OP.mult)
                nc.vector.tensor_tensor(sl, sl, tmp, op=OP.add)

            # ---------------- emission helpers -----------------------
            # Deferred-work queues: each entry is a closure emitting the
            # LN+rope (DVE + 1 small PE matmul) for one head chunk. They
            # are drained interleaved into the next phase's matmul stream
            # so the PE never stalls on DVE and the DVE FIFO never blocks
            # a PSUM-slot reuse.
            def kproj_pair(pr, drain):
                """k projection for 1024-token pair pr; returns (rsbs,
                deferred ln+rope closures). drain: list of closures to
                interleave (one per ec)."""
                tok0 = pr * 1024
                kslab = kslabs[pr]
                xb = proj.tile([P, DC, 1024], BF16, tag="xb", bufs=1,
                               name="xb")
                for d in range(DC):
                    nc.sync.dma_start(xb[:, d],
                                      xT[ds(d * P, P), ds(tok0, 1024)])
                cosk = proj.tile([HD, 1024], BF16, tag="cs2", bufs=4,
                                 name="cosk")
                nc.sync.dma_start(cosk, cosk_i[:, ds(tok0, 1024)])
                sink = proj.tile([HD, 1024], BF16, tag="cs2", bufs=4,
                                 name="sink")
                nc.sync.dma_start(sink, sink_i[:, ds(tok0, 1024)])
                pstats = [
                    psum.tile([1, QT], F32, tag="st", bufs=2, name="pstat_k")
                    for _ in range(2)
                ]
                for ec in range(NH):
                    w = proj.tile([P, DC, P], BF16, tag="w", bufs=4,
                                  name="wk")
                    nc.sync.dma_start(w, wqkT[NH + ec])
                    psk = psum.tile([P, 1024], F32, tag="pair", bufs=2,
                                    name="psk")
                    for d in range(DC):
                        for half in range(2):
                            nc.tensor.matmul(
                                psk[:, ds(half * QT, QT)], lhsT=w[:, d],
                                rhs=xb[:, d, ds(half * QT, QT)],
                                start=(d == 0), stop=(d == DC - 1),
                            )
                    nc.vector.tensor_copy(kslab[:, ec, :], psk)
                    sq = proj.tile([P, 1024], BF16, tag="sq", bufs=2,
                                   name="sqk")
                    nc.scalar.square(sq, psk)
                    for half in range(2):
                        nc.tensor.matmul(pstats[half], lhsT=ones_col,
                                         rhs=sq[:, ds(half * QT, QT)],
                                         start=(ec == 0), stop=(ec == NH - 1))
                    if ec < len(drain):
                        drain[ec]()
                rsbs = [stats_finish(pstats[half]) for half in range(2)]
                return xb, rsbs, cosk, sink

            def k_lnrope_closures(pr, rsbs, cosk, sink):
                kslab = kslabs[pr]
                out = []
                for ec in range(NH):
                    def fn(ec=ec):
                        for half in range(2):
                            sl = kslab[:, ec, ds(half * QT, QT)]
                            ln_apply(sl, rsbs[half], gk, bk, ec)
                            rope_apply(sl, cosk[:, ds(half * QT, QT)],
                                       sink[:, ds(half * QT, QT)])
                    out.append(fn)
                return out

            def vproj_pair(pr, xb, drain):
                """v projection for pair pr using resident xb; writes vts
                (token-major) via per-head feature-major matmuls + PE block
                transposes. drain: closures interleaved per head."""
                for ecv in range(NH):
                    wvc = proj.tile([P, DC, P], BF16, tag="w", bufs=4,
                                    name="wvc")
                    nc.sync.dma_start(wvc, wvTt[ecv])
                    psvf = psum.tile([P, 1024], F32, tag="pair", bufs=2,
                                     name="psvf")
                    for d in range(DC):
                        for half in range(2):
                            nc.tensor.matmul(
                                psvf[:, ds(half * QT, QT)], lhsT=wvc[:, d],
                                rhs=xb[:, d, ds(half * QT, QT)],
                                start=(d == 0), stop=(d == DC - 1),
                            )
                    vfsb = proj.tile([P, 1024], BF16, tag="vsb", bufs=3,
                                     name="vfsb")
                    nc.vector.tensor_copy(vfsb, psvf)
                    vtsb = proj.tile([P, 8, HD], BF16, tag="vtsb", bufs=2,
                                     name="vtsb")
                    for tc8 in range(8):
                        pst = psum.tile([P, P], BF16, tag="st", bufs=2,
                                        name="pst")
                        nc.tensor.transpose(pst, vfsb[:, ds(tc8 * P, P)],
                                            ident)
                        nc.vector.tensor_copy(vtsb[:, tc8], pst)
                    nc.sync.dma_start(
                        vts[ecv, ds(pr * 1024, 1024), :].rearrange(
                            "(kc p) hd -> p kc hd", p=P
                        ),
                        vtsb,
                    )
                    for di in (2 * ecv, 2 * ecv + 1):
                        if di < len(drain):
                            drain[di]()

            def qproj_tile(t, drain):
                """q projection for 512-token tile t into holdq; stats
                finished; returns (holdq, rsb)."""
                xq = proj.tile([P, DC, QT], BF16, tag="xq", bufs=1,
                               name="xq")
                for d in range(DC):
                    nc.sync.dma_start(xq[:, d],
                                      xTq[ds(d * P, P), ds(t * QT, QT)])
                cosq = proj.tile([HD, QT], BF16, tag="cs", bufs=4,
                                 name="cosq")
                nc.sync.dma_start(cosq, cosq_i[:, ds(t * QT, QT)])
                sinq = proj.tile([HD, QT], BF16, tag="cs", bufs=4,
                                 name="sinq")
                nc.sync.dma_start(sinq, sinq_i[:, ds(t * QT, QT)])
                holdq = proj.tile([P, NH, QT], BF16, tag="holdq", bufs=2,
                                  name="holdq")
                pstat = psum.tile([1, QT], F32, tag="st", bufs=2,
                                  name="pstat_q")
                for ec in range(NH):
                    w = proj.tile([P, DC, P], BF16, tag="w", bufs=4,
                                  name="wq")
                    nc.sync.dma_start(w, wqkT[ec])
                    psq = psum.tile([P, QT], F32, tag="sc", bufs=2,
                                    name="psq")
                    for d in range(DC):
                        nc.tensor.matmul(psq, lhsT=w[:, d], rhs=xq[:, d],
                                         start=(d == 0), stop=(d == DC - 1))
                    nc.vector.tensor_copy(holdq[:, ec], psq)
                    sq = proj.tile([P, QT], BF16, tag="sq", bufs=2,
                                   name="sqq")
                    nc.scalar.square(sq, psq)
                    nc.tensor.matmul(pstat, lhsT=ones_col, rhs=sq,
                                     start=(ec == 0), stop=(ec == NH - 1))
                    if ec < len(drain):
                        drain[ec]()
                rsb = stats_finish(pstat)
                return holdq, rsb, cosq, sinq

            def q_lnrope_closures(t, holdq, rsb, cosq, sinq):
                out = []
                for ec in range(NH):
                    def fn(ec=ec, last=(ec == NH - 1)):
                        sl = holdq[:, ec]
                        ln_apply(sl, rsb, gq, bq, ec)
                        rope_apply(sl, cosq, sinq)
                        if last:
                            nc.sync.dma_start(
                                qts[t].rearrange("h p q -> p h q"), holdq
                            )
                    out.append(fn)
                return out

            # ---------------- projection schedule ---------------------
            # each phase's LN/rope DVE work drains interleaved into the
            # next phase's matmul stream
            hq0, rsbq0, cq0, sq0 = qproj_tile(0, drain=[])
            lnq0 = q_lnrope_closures(0, hq0, rsbq0, cq0, sq0)
            xb0, rsbs0, ck0, sk0 = kproj_pair(0, drain=lnq0)
            ln0 = k_lnrope_closures(0, rsbs0, ck0, sk0)
            vproj_pair(0, xb0, drain=ln0)

            hq1, rsbq1, cq1, sq1 = qproj_tile(1, drain=[])
            lnq1 = q_lnrope_closures(1, hq1, rsbq1, cq1, sq1)
            xb1, rsbs1, ck1, sk1 = kproj_pair(1, drain=lnq1)
            ln1 = k_lnrope_closures(1, rsbs1, ck1, sk1)
            vproj_pair(1, xb1, drain=ln1)

            proj_ctx.close()

            # ============ attention + out-projection ===========
            attn_ctx = contextlib.ExitStack()
            attn = attn_ctx.enter_context(tc.tile_pool(name="attn", bufs=1))
            ot = attn.tile([P, NH, NQTOK], BF16, tag="ot", name="ot")
            PRE = 2  # score pair lookahead

            for t in range(NQ):
                mt = attn.tile([P, MAXM, QT], BF16, tag="masks", bufs=2,
                               name="mt")
                nc.sync.dma_start(mt, masks_i[t])
                mpos = {kc: i for i, kc in enumerate(masked[t])}
                n_slots = slots[t]
                n_pairs = n_slots // 2
                for h in range(NH):
                    qsl = attn.tile([P, QT], BF16, tag="qslab", bufs=3,
                                    name="qsl")
                    nc.sync.dma_start(qsl, qts[t, h])
                    vsl = attn.tile([P, KC, HD], BF16, tag="vslab", bufs=3,
                                    name="vsl")
                    nc.sync.dma_start(
                        vsl[:, :n_slots],
                        vts[h, ds(0, n_slots * P), :].rearrange(
                            "(kc p) hd -> p kc hd", p=P
                        ),
                    )
                    psout = psum.tile([P, QT], F32, tag="sc", bufs=2,
                                      name="psout")
                    dacc = attn.tile([P, 1024], BF16, tag="dacc", bufs=2,
                                     name="dacc")

                    ps_pairs = {}

                    def issue_scores(p, h=h, qsl=qsl, ps_pairs=ps_pairs):
                        pp = psum.tile([P, 1024], F32, tag="pair", bufs=2,
                                       name="pp")
                        for half in range(2):
                            kc = 2 * p + half
                            nc.tensor.matmul(
                                pp[:, ds(half * QT, QT)],
                                lhsT=kslabs[kc // 8][:, h,
                                                    ds((kc % 8) * P, P)],
                                rhs=qsl,
                            )
                        ps_pairs[p] = pp

                    for p in range(min(PRE, n_pairs)):
                        issue_scores(p)
                    for p in range(n_pairs):
                        if p + PRE < n_pairs:
                            issue_scores(p + PRE)
                        pp = ps_pairs.pop(p)
                        et2 = attn.tile([P, 1024], BF16, tag="exp", bufs=3,
                                        name="et2")
                        nc.scalar.activation(et2, pp, AF.Exp, bias=nege)
                        for half in range(2):
                            kc = 2 * p + half
                            if kc in mpos:
                                nc.vector.tensor_tensor(
                                    et2[:, ds(half * QT, QT)],
                                    et2[:, ds(half * QT, QT)],
                                    mt[:, mpos[kc]], op=OP.mult,
                                )
                        for half in range(2):
                            kc = 2 * p + half
                            nc.tensor.matmul(
                                psout,
                                lhsT=vsl[:, kc],
                                rhs=et2[:, ds(half * QT, QT)],
                                start=(kc == 0), stop=(kc == n_slots - 1),
                            )
                        # denominator accumulates on DVE, off the PE
                        if p == 0:
                            nc.vector.tensor_copy(dacc, et2)
                        else:
                            nc.vector.tensor_tensor(dacc, dacc, et2,
                                                    op=OP.add)

                    psden = psum.tile([1, QT], F32, tag="st", bufs=2,
                                      name="psden")
                    for half in range(2):
                        nc.tensor.matmul(psden, lhsT=ones_col,
                                         rhs=dacc[:, ds(half * QT, QT)],
                                         start=(half == 0), stop=(half == 1))
                    rec0 = attn.tile([1, QT], F32, tag="stats_sb", bufs=4,
                                     name="rec0")
                    with nc.allow_low_precision(
                        reason="denominator reciprocal, 18 bits is plenty"
                    ):
                        nc.vector.reciprocal_approx_fast(rec0, psden)
                    rec = attn.tile([1, QT], BF16, tag="stats_sb", bufs=4,
                                    name="rec")
                    nc.scalar.activation(rec, rec0, AF.Copy)
                    psr = psum.tile([P, QT], F32, tag="sc", bufs=2,
                                    name="psr")
                    nc.tensor.matmul(psr, lhsT=ones_row, rhs=rec)
                    rsb = attn.tile([P, QT], BF16, tag="rsbn", bufs=2,
                                    name="rsbn")
                    nc.vector.tensor_copy(rsb, psr)
                    nc.vector.tensor_tensor(ot[:, h, ds(t * QT, QT)], psout,
                                            rsb, op=OP.mult)

            # ---- out-projection over all q tokens ----
            for ec in range(DC):
                woc = attn.tile([P, NH, P], BF16, tag="woc", bufs=3,
                                name="woc")
                nc.sync.dma_start(woc, woTt[ec])
                psf = psum.tile([P, 1024], F32, tag="pair", bufs=2,
                                name="psf")
                for h in range(NH):
                    for half in range(2):
                        nc.tensor.matmul(
                            psf[:, ds(half * QT, QT)], lhsT=woc[:, h],
                            rhs=ot[:, h, ds(half * QT, QT)],
                            start=(h == 0), stop=(h == NH - 1),
                        )
                fsb = attn.tile([P, 1024], F32, tag="fsb", bufs=2, name="fsb")
                nc.vector.tensor_copy(fsb, psf)
                nc.sync.dma_start(out_t[ds(ec * P, P), :], fsb)

            attn_ctx.close()

    nc.compile()
    return nc


# --------------------------------------------------------------------------
# Host-side prep and driver
# --------------------------------------------------------------------------

def _q_blocks(role):
    """q-block indices (each 512 tokens) for a core role."""
    return [0, 3] if role == 0 else [1, 2]


def make_host_data(x, w_in, w_out, q_gamma, q_beta, k_gamma, k_beta, cfg,
                   n_cores=None):
    """Build per-core in_maps (list of dicts) + assembly metadata."""
    import ml_dtypes

    BF = ml_dtypes.bfloat16
    D = cfg["D"]
    S = cfg["S"]
    NQTOK = cfg["NQTOK"]
    QT = cfg["QT"]
    masked = cfg["masked"]
    NH = D // HD
    DC = D // P
    NQ = NQTOK // QT
    MAXM = max(len(m) for m in masked)
    B = x.shape[0]
    if n_cores is None:
        n_cores = 2 * B

    w64 = np.asarray(w_in, np.float64)
    wq = w64[0:D]
    wk = w64[D:2 * D]
    wv = w64[2 * D:3 * D]
    wq_c = wq - wq.mean(axis=0, keepdims=True)
    wk_c = wk - wk.mean(axis=0, keepdims=True)
    wqkT2 = np.concatenate([wq_c.T, wk_c.T], axis=1).astype(BF)
    # pre-tile to [2*NH, P, DC, P]: tile ec -> [p, dc, e] with contiguous rows
    wqkT = np.ascontiguousarray(
        wqkT2.reshape(DC, P, 2 * NH, P).transpose(2, 1, 0, 3)
    )
    wvTt = np.ascontiguousarray(
        wv.T.astype(BF).reshape(DC, P, NH, P).transpose(2, 1, 0, 3)
    )
    woT = np.asarray(w_out, np.float64).T  # [feat, e]
    # [ec, p(feat within head), h, e-col]
    woTt = np.ascontiguousarray(
        woT.reshape(NH, P, DC, P).transpose(2, 1, 0, 3).astype(BF)
    )

    inv = 1.0 / (10000.0 ** (np.arange(0, HD, 2, dtype=np.float64) / HD))
    tpos = np.arange(S, dtype=np.float64)
    fr = np.outer(tpos, inv)
    emb = np.concatenate([fr, fr], axis=-1)  # [S, HD]
    cosT = np.cos(emb).T  # [HD, S]
    sinTn = np.sin(emb).T

    # signed rotate-half permutation, as matmul lhsT:
    # out[p] = sum_{p'} rotmT[p', p] * in[p'] = rot_half(in)[p]
    h2 = HD // 2
    rotmT = np.zeros((P, P), np.float32)
    for p in range(h2):
        rotmT[p + h2, p] = -1.0
    for p in range(h2, HD):
        rotmT[p - h2, p] = 1.0

    scale = 1.0 / math.sqrt(HD)
    gq_a = np.ascontiguousarray(
        (np.asarray(q_gamma, np.float64) * scale).reshape(NH, P).T
    ).astype(np.float32)
    bq_a = np.ascontiguousarray(
        (np.asarray(q_beta, np.float64) * scale).reshape(NH, P).T
    ).astype(np.float32)
    gk_a = np.ascontiguousarray(
        np.asarray(k_gamma, np.float32).reshape(NH, P).T
    )
    bk_a = np.ascontiguousarray(
        np.asarray(k_beta, np.float32).reshape(NH, P).T
    )

    cosk = np.ascontiguousarray(cosT.astype(BF))
    sink = np.ascontiguousarray(sinTn.astype(BF))

    in_maps = []
    meta = []
    cores_per_batch = max(1, n_cores // B)
    for c in range(n_cores):
        b = c // cores_per_batch
        r = c % cores_per_batch
        blocks = _q_blocks(r)
        qtok = np.concatenate(
            [np.arange(bk * 512, (bk + 1) * 512) for bk in blocks]
        )
        xb = np.asarray(x[b], np.float32)  # [S, D]
        xTf = xb.T.astype(BF)              # [D, S]
        xT = np.ascontiguousarray(xTf)
        xTq = np.ascontiguousarray(xTf[:, qtok])
        cosq = np.ascontiguousarray(cosT[:, qtok].astype(BF))
        sinq = np.ascontiguousarray(sinTn[:, qtok].astype(BF))

        masks = np.zeros([NQ, P, MAXM, QT], np.float32)
        for t in range(NQ):
            q_start = blocks[t] * 512
            qq = np.arange(QT)
            kk = np.arange(P)
            for mi, kc in enumerate(masked[t]):
                masks[t, :, mi, :] = (
                    (kc * P + kk[:, None]) <= (q_start + qq[None, :])
                ).astype(np.float32)
        masks = masks.astype(BF)

        in_maps.append(dict(
            xTq=xTq, xT=xT, wqkT=wqkT, wvTt=wvTt, woTt=woTt,
            cosq=cosq, sinqn=sinq, cosk=cosk, sinkn=sink,
            gq=gq_a, bq=bq_a, gk=gk_a, bk=bk_a, masks=masks,
            rotm=rotmT.astype(BF),
            ident=np.eye(P, dtype=np.float32).astype(BF),
            onesc=np.ones((P, 1), BF),
            onesr=np.ones((1, P), BF),
        ))
        meta.append(dict(b=b, qtok=qtok))
    return in_maps, meta


_PROGRAM_CACHE = {}


def _get_program(cfg_key, cfg):
    if cfg_key not in _PROGRAM_CACHE:
        _PROGRAM_CACHE[cfg_key] = build_program(cfg)
    return _PROGRAM_CACHE[cfg_key]


def run_full(x, w_in, w_out, q_gamma, q_beta, k_gamma, k_beta,
             trace=False):
    from concourse.bass_utils import run_bass_kernel_spmd

    cfg = FULL_CFG
    B = x.shape[0]
    n_cores = 2 * B
    in_maps, meta = make_host_data(
        x, w_in, w_out, q_gamma, q_beta, k_gamma, k_beta, cfg,
        n_cores=n_cores,
    )
    nc = _get_program("full", cfg)
    res = run_bass_kernel_spmd(
        nc, in_maps, core_ids=list(range(n_cores)), trace=trace,
    )
    S, D = cfg["S"], cfg["D"]
    out = np.empty((B, S, D), np.float32)
    for c in range(n_cores):
        o = res.results[c]["out"]  # [D, NQTOK]
        out[meta[c]["b"], meta[c]["qtok"], :] = o.T
    return out, res


def kernel(x, w_in, w_out, q_gamma, q_beta, k_gamma, k_beta, n_heads=16,
           **_ignored):
    x = np.asarray(x, np.float32)
    assert int(np.asarray(n_heads)) * HD == x.shape[-1]
    out, _ = run_full(
        np.asarray(x, np.float32),
        np.asarray(w_in, np.float32),
        np.asarray(w_out, np.float32),
        np.asarray(q_gamma, np.float32),
        np.asarray(q_beta, np.float32),
        np.asarray(k_gamma, np.float32),
        np.asarray(k_beta, np.float32),
    )
    return out
